# revision 1
# baseline (speedup 1.0000x reference)
"""Trainium2 Bass kernel for nn_Concat_Model_89343909692135.

Computes out[b,i,j] = sigmoid(w_b.x1[b,i] + w_a.x1[b,j] + bias) for
B=2, N=4096, F=320, distributed over 8 NeuronCores.

Sharding: core k handles batch b = k//4, row block m = k%4 (1024 rows).
Each core receives its batch's x1 rolled so its own 1024 rows come
first (the SPMD program is identical across cores; only data differs),
and writes its output block TRANSPOSED: out_t[j, i] with j = all 4096
(rolled) column nodes on the partition axis and i = the core's 1024
own rows on the free axis. The host un-rolls and transposes back.

Why transposed: the j-dependent term p_j = x1[j].w_a lands naturally
in partition layout from the DVE dot products and feeds the Sigmoid
activation's per-partition bias directly — no broadcast needed. Only
the i-dependent term p_i (1024 values) must be broadcast across
partitions, and that happens ONCE per core (PE transpose + masked
ones-matmul), not once per chunk.

Device program:
  - hoisted x1 loads (8 SWDGE DMAs) keep the DMA engines busy while
    compute ramps.
  - B_i[p, i] = p_i[i] + conv_b, built once: DVE dots for the own 8
    row tiles -> [128,8], PE transpose -> [8,128], mask with the 8x8
    identity into a block-diagonal [8,8,128], one K=8 ones-matmul per
    PSUM bank -> [128,1024], copied to SBUF with the conv_b add folded
    in.
  - per 128-j tile (32 total): DVE dot -> p_j tile [128,1], one
    Sigmoid activation out = sigmoid(B_i + bias=p_j) on ScalarE, one
    fully-contiguous 512 KB store on the sync HWDGE queue.
"""

import numpy as np

import concourse.bass as bass
import concourse.mybir as mybir
import concourse.tile as tile
from concourse import bass_utils

B = 2
N = 4096
F = 320
P = 128
N_CORES = 8
BLOCKS_PER_BATCH = N_CORES // B  # 4
ROWS_PER_CORE = N // BLOCKS_PER_BATCH  # 1024
ROW_TILES = ROWS_PER_CORE // P  # 8
COL_TILES = N // P  # 32
LOAD_GROUP = 4  # column tiles per load DMA
BANK = 512  # fp32 elements per PSUM bank


def _split_multiwait_instructions(nc):
    # The walrus build here only accepts one sem-wait per instruction.
    # Hoist extra waits onto preceding NoOps on the same engine queue;
    # in-order execution per engine makes this equivalent.
    seen_dma = False
    for fn in nc.m.functions:
        for bb in fn.blocks:
            new_list = []
            for ins in bb.instructions:
                # strip the all-engine ENTRY barrier (drain + EVSEM
                # butterfly before any real work): engines enter with
                # clean state (the exit sequence cleared sems) and all
                # real cross-engine deps are explicit Tile semaphores
                nm = type(ins).__name__
                if nm == "InstDMACopy":
                    seen_dma = True
                if not seen_dma and nm in ("InstDrain", "InstEventSemaphore"):
                    continue
                # drop the framework's unused const-tile memsets (the
                # verifier flags them as having no reader); they sit at
                # the head of the Pool queue and delay the first x1
                # load emission
                if (
                    type(ins).__name__ == "InstMemset"
                    and ins.outs
                    and getattr(ins.outs[0], "memref", "")
                    in (
                        "const-float32-0.0",
                        "const-float32-1.0",
                        "const-bfloat16-1.0",
                        "const-uint8-127",
                    )
                ):
                    continue
                si = getattr(ins, "sync_info", None)
                if si is not None and si.on_wait and len(si.on_wait) > 1:
                    waits = list(si.on_wait)
                    for i, w in enumerate(waits[:-1]):
                        nop = mybir.InstNoOp(
                            name=f"{ins.name}-w{i}",
                            ins=[],
                            outs=[],
                            engine=ins.engine,
                            sync_info=type(si)(on_wait=[w], on_update=[]),
                        )
                        new_list.append(nop)
                    si.on_wait = waits[-1:]
                new_list.append(ins)
            bb.instructions[:] = new_list


def _build_program(fixup=True):
    nc = bass.Bass("TRN2", debug=False, target_bir_lowering=False)
    f32 = mybir.dt.float32
    x_d = nc.dram_tensor("x1r", [N, F], f32, kind="ExternalInput").ap()
    w_d = nc.dram_tensor("conv_w", [2 * F], f32, kind="ExternalInput").ap()
    b_d = nc.dram_tensor("conv_b", [1], f32, kind="ExternalInput").ap()
    id_d = nc.dram_tensor("ident", [P, P], f32, kind="ExternalInput").ap()
    on_d = nc.dram_tensor("ones", [ROW_TILES, P], f32, kind="ExternalInput").ap()
    o_d = nc.dram_tensor("out", [N, ROWS_PER_CORE], f32, kind="ExternalOutput").ap()

    with tile.TileContext(nc) as tc:
        with (
            tc.tile_pool(name="singles", bufs=1) as singles,
            tc.tile_pool(name="xpool", bufs=1) as xpool,
            tc.tile_pool(name="small", bufs=2) as small,
            tc.tile_pool(name="outp", bufs=8) as outp,
            tc.tile_pool(name="psum", bufs=1, space="PSUM") as psum,
            tc.tile_pool(name="pst", bufs=1, space="PSUM") as pst,
        ):
            w_rep = singles.tile([P, 2 * F], f32)
            b_bcast = singles.tile([P, 1], f32)
            ident = singles.tile([P, P], f32)
            ones_k = singles.tile([ROW_TILES, P], f32)
            # w_b half first: it gates the very first p_i dot
            nc.sync.dma_start(
                out=w_rep[:, F : 2 * F], in_=w_d[F : 2 * F].partition_broadcast(P)
            )
            nc.sync.dma_start(
                out=w_rep[:, 0:F], in_=w_d[0:F].partition_broadcast(P)
            )
            nc.sync.dma_start(out=b_bcast, in_=b_d.partition_broadcast(P))
            nc.sync.dma_start(out=ident, in_=id_d)
            nc.sync.dma_start(out=ones_k, in_=on_d)
            w_a_rep = w_rep[:, 0:F]
            w_b_rep = w_rep[:, F : 2 * F]

            # warm-up: trigger the sigmoid ACT-table load (~2.7us on
            # real HW, invisible to the cost model) while x1 streams in
            warm = singles.tile([P, 1], f32)
            nc.scalar.activation(
                out=warm,
                in_=b_bcast,
                func=mybir.ActivationFunctionType.Sigmoid,
                bias=b_bcast[:, 0:1],
            )

            # hoisted x1 loads on the SWDGE (gpsimd) queue. Group 0 is
            # split into two 2-tile DMAs: shorter Q7 descriptor
            # emission, so the first transfer (and the whole B_i chain
            # behind it) starts ~1us earlier. Remaining groups are 4
            # tiles (656 KB) each.
            xt0a = xpool.tile([P, 2, F], f32, name="xt0a", tag="xt0a", bufs=1)
            nc.gpsimd.dma_start(
                out=xt0a, in_=x_d[0 : 2 * P, :].rearrange("(t p) f -> p t f", p=P)
            )
            xt0b = xpool.tile([P, 2, F], f32, name="xt0b", tag="xt0b", bufs=1)
            nc.gpsimd.dma_start(
                out=xt0b,
                in_=x_d[2 * P : 4 * P, :].rearrange("(t p) f -> p t f", p=P),
            )
            xts = [None]
            for g in range(1, COL_TILES // LOAD_GROUP):
                xt = xpool.tile(
                    [P, LOAD_GROUP, F], f32, name=f"xt{g}", tag=f"xt{g}", bufs=1
                )
                src = x_d[
                    g * LOAD_GROUP * P : (g + 1) * LOAD_GROUP * P, :
                ].rearrange("(t p) f -> p t f", p=P)
                nc.gpsimd.dma_start(out=xt, in_=src)
                xts.append(xt)

            def col_tile(j):
                if j < 2:
                    return xt0a[:, j, :]
                if j < 4:
                    return xt0b[:, j - 2, :]
                return xts[j // LOAD_GROUP][:, j % LOAD_GROUP, :]

            # B_i = p_i + conv_b, broadcast across partitions (once).
            # Own rows are column tiles 0..7 thanks to the roll. One
            # batched mul+reduce per 4-tile load group.
            w_b_g = bass.AP(
                tensor=w_rep.tensor,
                offset=w_b_rep.offset,
                ap=[w_rep.ap[0], [0, LOAD_GROUP], [1, F]],
            )
            w_a_g = bass.AP(
                tensor=w_rep.tensor,
                offset=w_a_rep.offset,
                ap=[w_rep.ap[0], [0, LOAD_GROUP], [1, F]],
            )
            w_b_g2 = bass.AP(
                tensor=w_rep.tensor,
                offset=w_b_rep.offset,
                ap=[w_rep.ap[0], [0, 2], [1, F]],
            )
            bi_sb = singles.tile([P, ROWS_PER_CORE], f32)
            HG = ROW_TILES // 2  # 4 row tiles per half-chain
            for h in range(2):
                # independent half-chain: gated only by its own 4-tile
                # dot group, so the first activations start early
                pib = small.tile([P, HG], f32, name=f"pib{h}", tag="pib", bufs=2)
                if h == 0:
                    # two 2-tile pairs matching the split group-0 loads
                    for q, xh in enumerate((xt0a, xt0b)):
                        scr = small.tile(
                            [P, 2, F], f32, name=f"scri0{q}", tag="scr2", bufs=2
                        )
                        nc.vector.tensor_mul(out=scr, in0=xh, in1=w_b_g2)
                        nc.vector.reduce_sum(
                            out=pib[:, q * 2 : (q + 1) * 2],
                            in_=scr,
                            axis=mybir.AxisListType.X,
                        )
                else:
                    scr = small.tile(
                        [P, HG, F], f32, name=f"scri{h}", tag="scrg", bufs=2
                    )
                    nc.vector.tensor_mul(out=scr, in0=xts[h], in1=w_b_g)
                    nc.vector.reduce_sum(
                        out=pib, in_=scr, axis=mybir.AxisListType.X
                    )

                piT_ps = pst.tile([HG, P], f32, name=f"piTps{h}", tag="piTps", bufs=2)
                nc.tensor.transpose(piT_ps, pib, ident)
                piT = small.tile([HG, P], f32, name=f"piT{h}", tag="piT", bufs=2)
                nc.vector.tensor_copy(out=piT, in_=piT_ps)

                rhs = small.tile(
                    [HG, HG, P], f32, name=f"rhs{h}", tag="rhs", bufs=2
                )
                piT_b = bass.AP(
                    tensor=piT.tensor,
                    offset=piT.offset,
                    ap=[piT.ap[0], [0, HG], piT.ap[1]],
                )
                identh_b = bass.AP(
                    tensor=ident.tensor,
                    offset=ident.offset,
                    ap=[[ident.ap[0][0], HG], [ident.ap[1][0], HG], [0, P]],
                )
                nc.vector.tensor_tensor(
                    out=rhs, in0=piT_b, in1=identh_b, op=mybir.AluOpType.mult
                )

                bch = psum.tile([P, BANK], f32, name=f"bc{h}", tag=f"bc{h}", bufs=1)
                nc.tensor.matmul(
                    bch,
                    ones_k[0:HG, :],
                    rhs,
                    start=True,
                    stop=True,
                )
                # PSUM -> SBUF copy with the conv_b add folded in
                nc.vector.tensor_scalar_add(
                    out=bi_sb[:, h * BANK : (h + 1) * BANK],
                    in0=bch,
                    scalar1=b_bcast[:, 0:1],
                )

            # main loop: one dot, one activation, one fully-contiguous
            # 512 KB store per j tile (fine granularity keeps the DVE
            # ahead of the ScalarEngine's activation stream)
            for j in range(COL_TILES):
                scr = small.tile([P, F], f32, name=f"scrj{j}", tag="scr", bufs=4)
                pjv = small.tile([P, 1], f32, name=f"pjv{j}", tag="pjv", bufs=4)
                nc.vector.tensor_mul(out=scr, in0=col_tile(j), in1=w_a_rep)
                nc.vector.reduce_sum(
                    out=pjv, in_=scr, axis=mybir.AxisListType.X
                )
                ot = outp.tile(
                    [P, ROWS_PER_CORE], f32, name=f"ot{j}", tag="ot", bufs=8
                )
                nc.scalar.activation(
                    out=ot,
                    in_=bi_sb,
                    func=mybir.ActivationFunctionType.Sigmoid,
                    bias=pjv,
                    scale=1.0,
                )
                nc.sync.dma_start(
                    out=o_d[j * P : (j + 1) * P, :],
                    in_=ot,
                )

    if fixup:
        _split_multiwait_instructions(nc)
    return nc


_NC = None


def _get_program():
    global _NC
    if _NC is None:
        _NC = _build_program()
    return _NC


def _run_spmd(x1, conv_w, conv_b, trace=False, **run_kwargs):
    x1 = np.ascontiguousarray(x1, dtype=np.float32)
    conv_w = np.ascontiguousarray(conv_w, dtype=np.float32)
    conv_b = np.ascontiguousarray(conv_b, dtype=np.float32)
    ident = np.eye(P, dtype=np.float32)
    ones = np.ones((ROW_TILES, P), dtype=np.float32)

    nc = _get_program()
    in_maps = []
    for k in range(N_CORES):
        b, m = divmod(k, BLOCKS_PER_BATCH)
        x1r = np.ascontiguousarray(np.roll(x1[b], -ROWS_PER_CORE * m, axis=0))
        in_maps.append(
            {
                "x1r": x1r,
                "conv_w": conv_w,
                "conv_b": conv_b,
                "ident": ident,
                "ones": ones,
            }
        )

    res = bass_utils.run_bass_kernel_spmd(
        nc, in_maps, core_ids=list(range(N_CORES)), trace=trace, **run_kwargs
    )

    out = np.empty((B, N, N), dtype=np.float32)
    for k in range(N_CORES):
        b, m = divmod(k, BLOCKS_PER_BATCH)
        blk = res.results[k]["out"]  # [N(j, rolled), ROWS_PER_CORE(i)]
        out[b, m * ROWS_PER_CORE : (m + 1) * ROWS_PER_CORE, :] = np.roll(
            blk, ROWS_PER_CORE * m, axis=0
        ).T
    return out, res


def kernel(x1, conv_w, conv_b):
    return _run_spmd(x1, conv_w, conv_b)[0]



# revision 25
# speedup vs baseline: 1.3838x; 1.3838x over previous
"""Trainium2 Bass kernel for nn_Concat_Model_89343909692135.

Computes out[b,i,j] = sigmoid(w_b.x1[b,i] + w_a.x1[b,j] + bias) for
B=2, N=4096, F=320, distributed over 8 NeuronCores.

Sharding: 2x2x2 blocking. Core k handles batch b = k//4 and the
(i-half, j-half) quadrant q = k%4 of that batch's 4096x4096 output:
a 2048(i) x 2048(j) block. The device writes its block TRANSPOSED:
out_t[j, i] with j on the partition axis (16 tiles of 128) and i on
the free axis (2048 wide). The host transposes back.

Memory-regime design (baseline was 65.6us, fp32 everywhere):
  - fp16 inputs (host cast; ~7e-3 rel err vs 2e-2 budget), bf16
    output (wide exponent covers tiny sigmoids; host upcasts).
    Total DMA per core: ~22 MB -> ~11 MB.
  - p_i dots run on the otherwise-idle PE: the host supplies xiT
    (f-major), so each 512-row chunk is 3 accumulating [K<=128,1]x
    [K,512] matmuls producing p_i as a [1, 512] PSUM row -- exactly
    the rhs layout the ones-matmul broadcast needs. +conv_b and the
    fp16 round happen in one DVE op on the [1,512] row.
  - B_i chunks live in per-chunk PSUM tiles; the ACT ramp reads
    those directly (PSUM dep-tracking is whole-tile, so per-chunk
    tiles give the right granularity), while a lazy DVE copy builds
    the contiguous SBUF B_i for the 2048-wide steady-state ACTs.
  - p_j dots (bias per j tile) are one scalar_tensor_tensor with
    accum_out per tile on the DVE.
  - The ACT stream is an explicit piece schedule: narrow ramp pieces
    chase B_i chunk completion, then 2048-wide instructions; the
    final tile is split so the kernel doesn't tail-stall on a full
    512KB store.
"""

import numpy as np

import concourse.bass as bass
import concourse.mybir as mybir
import concourse.tile as tile
from concourse import bass_utils

B = 2
N = 4096
F = 320
P = 128
N_CORES = 8
R = 2048  # i extent per core (ACT free axis)
C = 2048  # j extent per core (partition tiles)
IT = R // P  # 16 i tiles
JT = C // P  # 16 j tiles
NCH = 4  # B_i chunks (512 cols each)
CHW = R // NCH  # 512
FP = 384  # f padded to 3*128 so each xiT chunk loads as ONE DMA

# --- schedule knobs -------------------------------------------------
XJ_GROUPS = [(0, 2), (2, 2), (4, 4), (8, 4), (12, 4)]  # SWDGE loads
# DVE emission order: ('ppb', c) = +b/fp16 round of p_i chunk c,
# ('cp', c) = PSUM->SBUF copy of B_i chunk c, ('jd', jt) = p_j dot.
DVE_ORDER = (
    [("jd", 0), ("jd", 1), ("cp", 0), ("jd", 2), ("cp", 1), ("jd", 3),
     ("cp", 2), ("jd", 4), ("cp", 3), ("jd", 5)]
    + [("jd", t) for t in range(6, JT)]
)
# ACT piece schedule: (jt, chunk) ramp pieces read PSUM chunk tiles;
# remaining tiles run 2048-wide from SBUF (auto-emitted), tail split.
RAMP = [
    (0, 0), (1, 0), (2, 0), (0, 1), (1, 1), (2, 1),
    (0, 2), (1, 2), (2, 2), (0, 3), (1, 3), (2, 3),
]
TAIL_JT = 15
TAIL_SPLIT = [1024, 512, 512]


def _split_multiwait_instructions(nc):
    # The walrus build here only accepts one sem-wait per instruction.
    # Hoist extra waits onto preceding NoOps on the same engine queue;
    # in-order execution per engine makes this equivalent.
    seen_dma = False
    for fn in nc.m.functions:
        for bb in fn.blocks:
            new_list = []
            for ins in bb.instructions:
                # strip the all-engine ENTRY barrier (drain + EVSEM
                # butterfly before any real work); real cross-engine
                # deps are explicit Tile semaphores
                nm = type(ins).__name__
                if nm == "InstDMACopy":
                    seen_dma = True
                if not seen_dma and nm in ("InstDrain", "InstEventSemaphore"):
                    continue
                # drop the framework's unused const-tile memsets
                if (
                    nm == "InstMemset"
                    and ins.outs
                    and getattr(ins.outs[0], "memref", "")
                    in (
                        "const-float32-0.0",
                        "const-float32-1.0",
                        "const-bfloat16-1.0",
                        "const-uint8-127",
                    )
                ):
                    continue
                si = getattr(ins, "sync_info", None)
                if si is not None and si.on_wait and len(si.on_wait) > 1:
                    waits = list(si.on_wait)
                    for i, w in enumerate(waits[:-1]):
                        nop = mybir.InstNoOp(
                            name=f"{ins.name}-w{i}",
                            ins=[],
                            outs=[],
                            engine=ins.engine,
                            sync_info=type(si)(on_wait=[w], on_update=[]),
                        )
                        new_list.append(nop)
                    si.on_wait = waits[-1:]
                new_list.append(ins)
            bb.instructions[:] = new_list


def _build_program(fixup=True, **opts):
    xj_groups = opts.get("xj_groups", XJ_GROUPS)
    dve_order = opts.get("dve_order", DVE_ORDER)
    ramp = opts.get("ramp", RAMP)
    tail_jt = opts.get("tail_jt", TAIL_JT)
    tail_split = opts.get("tail_split", TAIL_SPLIT)

    nc = bass.Bass("TRN2", debug=False, target_bir_lowering=False)
    f32 = mybir.dt.float32
    f16 = mybir.dt.float16
    bf16 = mybir.dt.bfloat16
    xiT_d = nc.dram_tensor("xiT", [FP, R], f16, kind="ExternalInput").ap()
    xj_d = nc.dram_tensor("xj", [C, F], f16, kind="ExternalInput").ap()
    w_d = nc.dram_tensor("conv_w", [2 * F], f16, kind="ExternalInput").ap()
    on_d = nc.dram_tensor("ones1", [1, P], f16, kind="ExternalInput").ap()
    wp_d = nc.dram_tensor("wb_pad", [FP], f16, kind="ExternalInput").ap()
    o_d = nc.dram_tensor("out", [C, R], bf16, kind="ExternalOutput").ap()

    with tile.TileContext(nc) as tc:
        with (
            tc.tile_pool(name="singles", bufs=1) as singles,
            tc.tile_pool(name="xpool", bufs=1) as xpool,
            tc.tile_pool(name="small", bufs=2) as small,
            tc.tile_pool(name="outp", bufs=1) as outp,
            tc.tile_pool(name="bips", bufs=1, space="PSUM") as bips,
            tc.tile_pool(name="pst", bufs=2, space="PSUM") as pst,
        ):
            w_rep = singles.tile([P, F], f16)  # w_a replicated (j dots)
            ones1 = singles.tile([1, P], f16)
            wbp = singles.tile([P, 3], f16)  # padded w_b (+conv_b) cols

            # sync queue: one DMA per xiT chunk (these gate B_i and
            # therefore every ACT piece), then the B_i broadcast DMAs.
            xiT_t = []
            for c in range(NCH):
                t = xpool.tile(
                    [P, 3, CHW], f16, name=f"xiT{c}", tag=f"xiT{c}", bufs=1
                )
                nc.sync.dma_start(
                    out=t,
                    in_=xiT_d[:, c * CHW : (c + 1) * CHW].rearrange(
                        "(t p) col -> p t col", p=P
                    ),
                )
                xiT_t.append(t)
            xj_t = [None] * JT
            for s, n in xj_groups:
                t = xpool.tile([P, n, F], f16, name=f"xj{s}", tag=f"xj{s}", bufs=1)
                nc.gpsimd.dma_start(
                    out=t,
                    in_=xj_d[s * P : (s + n) * P, :].rearrange(
                        "(t p) f -> p t f", p=P
                    ),
                )
                for k in range(n):
                    xj_t[s + k] = t[:, k, :]

            # scalar queue: weight tiles + ones (tiny; ACT is idle
            # until the table-load warm-up anyway). wbp holds w_b
            # padded to 384 with conv_b at index F (xiT's row F is
            # all-ones, so the dot lands p_i + conv_b directly).
            nc.scalar.dma_start(out=w_rep, in_=w_d[0:F].partition_broadcast(P))
            nc.scalar.dma_start(
                out=wbp, in_=wp_d.rearrange("(t p) -> p t", p=P)
            )
            nc.scalar.dma_start(out=ones1, in_=on_d)

            # warm-up: pay the sigmoid ACT-table load (~1.3us) early,
            # on a locally memset tile so it waits for no DMA
            wz = singles.tile([P, 1], f16)
            nc.vector.memset(wz, 0.0)
            warm = singles.tile([P, 1], f16)
            sig = mybir.ActivationFunctionType.Sigmoid
            nc.scalar.activation(
                out=warm, in_=wz, func=sig, bias=wz[:, 0:1]
            )

            mm = mybir.AluOpType.mult

            # PE warm-up: ~40 tiny dummy matmuls keep the PE busy from
            # t~0.5us so its p-state ramps toward full clock before
            # the real dot matmuls arrive (cold PE runs 2-4x slower)
            wps = pst.tile([1, P], f32, name="wps", tag="wps", bufs=1)
            wz16 = singles.tile([1, P], f16, name="wz16", tag="wz16", bufs=1)
            nc.vector.memset(wz16, 0.0)
            for wi in range(40):
                nc.tensor.matmul(wps, wz16[0:1, 0:1], wz16, start=True, stop=True)

            # B_i lives in PSUM twice: per-chunk tiles bic[0..1] give
            # the ramp chunk-granular deps (PSUM dep tracking is
            # whole-tile), and one [P, R] tile bi_ps collects ALL
            # chunks for the wide ACTs (which need every chunk anyway)
            bic = [
                bips.tile([P, CHW], f32, name=f"bic{c}", tag=f"bic{c}", bufs=1)
                for c in range(NCH)
            ]
            bi_sb = singles.tile([P, R], f32, name="bi", tag="bi", bufs=1)
            pjv = [None] * JT

            # Per chunk: 3 accumulating dot matmuls on the PE give the
            # p_i row [1, CHW] f32 (padding rows are zero except the
            # all-ones row that adds conv_b); DVE rounds it to an fp16
            # SBUF row; PE ones-matmuls broadcast it across all 128
            # partitions -- into bic[c] (ramp source, chunks 0/1) and
            # into bi_ps's chunk columns (wide-ACT source).
            for c in range(NCH):
                ppc = pst.tile([1, CHW], f32, name=f"pp{c}", tag="pp", bufs=2)
                for fi in range(3):
                    nc.tensor.matmul(
                        ppc,
                        wbp[:, fi : fi + 1],
                        xiT_t[c][:, fi, :],
                        start=(fi == 0),
                        stop=(fi == 2),
                    )
                ppbc = small.tile([1, CHW], f16, name=f"ppb{c}", tag=f"ppb{c}", bufs=1)
                nc.vector.tensor_copy(out=ppbc, in_=ppc)
                nc.tensor.matmul(bic[c], ones1, ppbc, start=True, stop=True)

            def emit_jdot(jt):
                pv = small.tile([P, 1], f32, name=f"pjv{jt}", tag=f"pjv{jt}", bufs=1)
                prod = small.tile(
                    [P, F], f16, name=f"pjprod{jt}", tag="prod", bufs=4
                )
                nc.vector.scalar_tensor_tensor(
                    out=prod,
                    in0=xj_t[jt],
                    scalar=1.0,
                    in1=w_rep,
                    op0=mm,
                    op1=mm,
                    accum_out=pv,
                )
                pjv[jt] = pv

            def emit_copy(c):
                nc.vector.tensor_copy(
                    out=bi_sb[:, c * CHW : (c + 1) * CHW], in_=bic[c]
                )

            for kind, idx in dve_order:
                if kind == "cp":
                    emit_copy(idx)
                else:
                    emit_jdot(idx)
            for jt in range(JT):
                if pjv[jt] is None:
                    emit_jdot(jt)

            # ---- ACT schedule ----
            ot_tiles = [None] * JT
            covered = [0] * JT

            def get_ot(jt):
                if ot_tiles[jt] is None:
                    ot_tiles[jt] = outp.tile(
                        [P, R], bf16, name=f"ot{jt}", tag=f"ot{jt}", bufs=1
                    )
                return ot_tiles[jt]

            store_q = [0]

            def store_full(jt):
                for h in range(2):
                    eng = nc.sync if store_q[0] % 2 == 0 else nc.gpsimd
                    store_q[0] += 1
                    eng.dma_start(
                        out=o_d[jt * P : (jt + 1) * P, h * 1024 : (h + 1) * 1024],
                        in_=ot_tiles[jt][:, h * 1024 : (h + 1) * 1024],
                    )

            for jt, c in ramp:
                ot = get_ot(jt)
                src_ap = bic[c]
                nc.scalar.activation(
                    out=ot[:, c * CHW : (c + 1) * CHW],
                    in_=src_ap,
                    func=sig,
                    bias=pjv[jt],
                    scale=1.0,
                )
                covered[jt] += CHW
                if covered[jt] == R:
                    store_full(jt)
            for jt in range(JT):
                if jt == tail_jt or covered[jt] == R:
                    continue
                assert covered[jt] == 0, "ramp must cover whole tiles"
                ot = get_ot(jt)
                nc.scalar.activation(
                    out=ot, in_=bi_sb, func=sig, bias=pjv[jt], scale=1.0
                )
                covered[jt] = R
                store_full(jt)
            # tail tile: piecewise ACT + store so the kernel doesn't
            # end waiting on a full-width store
            lo = covered[tail_jt]
            ot = get_ot(tail_jt)
            for wdt in tail_split:
                hi = min(lo + wdt, R)
                if hi <= lo:
                    break
                nc.scalar.activation(
                    out=ot[:, lo:hi],
                    in_=bi_sb[:, lo:hi],
                    func=sig,
                    bias=pjv[tail_jt],
                    scale=1.0,
                )
                eng = nc.sync if store_q[0] % 2 == 0 else nc.gpsimd
                store_q[0] += 1
                eng.dma_start(
                    out=o_d[tail_jt * P : (tail_jt + 1) * P, lo:hi],
                    in_=ot[:, lo:hi],
                )
                lo = hi

    if fixup:
        _split_multiwait_instructions(nc)
    return nc


_NC = None


def _get_program():
    global _NC
    if _NC is None:
        _NC = _build_program()
    return _NC


def _run_spmd(x1, conv_w, conv_b, trace=False, **run_kwargs):
    x16 = np.ascontiguousarray(x1, dtype=np.float16)
    w16 = np.ascontiguousarray(conv_w, dtype=np.float16)
    ones1 = np.ones((1, P), dtype=np.float16)
    wb_pad = np.zeros(FP, dtype=np.float16)
    wb_pad[:F] = w16[F:]
    wb_pad[F] = conv_b[0]

    xiTp = [[None, None], [None, None]]
    for b in range(B):
        for hi in range(2):
            m = np.zeros((FP, R), dtype=np.float16)
            m[:F] = x16[b, hi * R : (hi + 1) * R].T
            m[F] = 1.0
            xiTp[b][hi] = m

    nc = _get_program()
    in_maps = []
    for k in range(N_CORES):
        b, q = divmod(k, 4)
        hi, hj = divmod(q, 2)
        in_maps.append(
            {
                "xiT": xiTp[b][hi],
                "xj": np.ascontiguousarray(x16[b, hj * C : (hj + 1) * C]),
                "conv_w": w16,
                "ones1": ones1,
                "wb_pad": wb_pad,
            }
        )

    res = bass_utils.run_bass_kernel_spmd(
        nc, in_maps, core_ids=list(range(N_CORES)), trace=trace, **run_kwargs
    )

    out = np.empty((B, N, N), dtype=np.float32)
    for k in range(N_CORES):
        b, q = divmod(k, 4)
        hi, hj = divmod(q, 2)
        blk = res.results[k]["out"]  # [C(j), R(i)] bf16
        out[b, hi * R : (hi + 1) * R, hj * C : (hj + 1) * C] = blk.T.astype(
            np.float32
        )
    return out, res


def kernel(x1, conv_w, conv_b):
    return _run_spmd(x1, conv_w, conv_b)[0]
